# revision 67
# baseline (speedup 1.0000x reference)
"""Trainium2 Bass kernel for nn_MHAAttention (LayerNorm2d + MHA w/ rel-pos bias + residual).

Sharding: data-parallel over batch — 8 batch elements, one per NeuronCore.
No collectives needed.

Per-core pipeline (matmuls in bf16, accum fp32; residual path fp32):
  LN stats via ones-matmuls over host-sent bf16 x and x^2 (stats replicated
  across partitions by the M=128 ones stationary); rsqrt as exp(-0.5*ln(v+eps))
  so the Scalar engine stays in the one natural_log_exp table set.
  Q,K in (d part, t free); V in (t part, per-head [v|1] / [1|v] augment so the
  softmax denominator Z rides along the AV matmul and even/odd head outputs
  land on partitions 0-63 / 64-127 for pair-packed output projection.
  scores computed transposed per head (K=64 matmul), exp on ScalarE (scores
  are bounded, no max-subtraction), rel-pos bias applied multiplicatively:
  aT = exp(s) * exp(bias) with exp(bias) strips precomputed on host (bf16,
  2x-rate DVE multiply instead of a 1x PSUM add).
  Z inverted per head with reciprocal_approx_fast, replicated across
  partitions with a gpsimd partition_broadcast, multiplied on DVE.
  Output projection accumulates head-pairs (K=128), bias via K=1 ones-row
  matmul, residual add on DVE, DMA out per chunk.
"""

import sys

for _p in ("/opt/trn_rl_repo",):
    if _p not in sys.path:
        sys.path.insert(0, _p)

from contextlib import ExitStack

import ml_dtypes
import numpy as np

import concourse.bass as bass
import concourse.mybir as mybir
import concourse.tile as tile
from concourse import library_config
from concourse.bass_utils import run_bass_kernel_spmd

import os

USE_RECIP_FAST = os.environ.get("K_RECIP_FAST", "1") == "1"
# InstPartitionBroadcast fails walrus codegen ("ISA wrong length") on this
# build — default to the DMA 0-stride-source broadcast instead.
USE_PBCAST = os.environ.get("K_PBCAST", "0") == "1"
PROBE_NOEXP = os.environ.get("K_PROBE_NOEXP", "0") == "1"
FP8_QKV = os.environ.get("K_FP8QKV", "1") == "1"
# Normalize by Z with a DVE tensor_tensor divide on the raw broadcast Z
# (instead of 1/Z = exp(-ln Z) on ACT + multiply). NOTE: divide is not a
# valid DVE TT op on this HW (walrus s3s3d3_tt_valid_op) — keep 0.
USE_DIV = os.environ.get("K_DIV", "0") == "1"
# fp8 DoubleRow attention-value matmuls: aT and v in fp8e4m3, jt-pairs
# contracted as K=256. Halves the AV matmul count but measured neutral on HW
# (DoubleRow matmuls run no faster at the throttled attention-phase clock)
# with 6x worse rel err — keep off.
FP8_AV = os.environ.get("K_FP8AV", "0") == "1"

F32 = mybir.dt.float32
BF16 = mybir.dt.bfloat16
FP8 = mybir.dt.float8e4
AF = mybir.ActivationFunctionType
OP = mybir.AluOpType
NPBF = ml_dtypes.bfloat16
NPF8 = ml_dtypes.float8_e4m3
FP8_SCALE = 16.0  # weights x16 so fp8e4m3 sees normal-range values

B = 8
CH = 512
H = W = 32
NT = H * W          # 1024 tokens
HEADS = 8
HD = 64
HP = HEADS // 2     # head pairs
EPS = 1e-6
P = 128
CT = CH // P        # 4 channel tiles
TT = NT // P        # 8 token tiles
IC = NT // 512      # 2 free-dim chunks of 512
STRIP_W = 60 * 32   # 1920
VW = 128            # per-head v-aug width (padded so AV writes PSUM base 0)


def _build_strips(rel: np.ndarray) -> np.ndarray:
    """(3969, 8) rel table -> (8, 128, 1920) bias strips.

    strip[h, 32*jh_l + jw, 32*g + iw] = T_h[g - jh_l + 3, iw - jw + 31]
    where T_h = rel[:, h].reshape(63, 63).
    bias.T block for key-tile jt is then strip[:, (28-4*jt)*32 : +1024].
    """
    T = rel.reshape(63, 63, HEADS)  # [a, b, h]
    jh_l = np.arange(4)[:, None, None, None]
    jw = np.arange(32)[None, :, None, None]
    g = np.arange(60)[None, None, :, None]
    iw = np.arange(32)[None, None, None, :]
    a = g - jh_l + 3          # in [0,62]
    b = iw - jw + 31          # in [0,62]
    a_b, b_b = np.broadcast_arrays(a, b)
    out = T[a_b, b_b, :]      # (4, 32, 60, 32, 8)
    out = np.ascontiguousarray(np.moveaxis(out, -1, 0)).reshape(HEADS, 128, STRIP_W)
    return out.astype(np.float32)


def _build_nc() -> bass.Bass:
    nc = bass.Bass()

    x_d = nc.declare_dram_parameter("x", [CH, NT], F32, isOutput=False)
    xbf_d = nc.declare_dram_parameter("xbf", [CH, NT], BF16, isOutput=False)
    x2_d = nc.declare_dram_parameter("x2", [CH, NT], BF16, isOutput=False)
    wdt = FP8 if FP8_QKV else BF16
    wqT_d = nc.declare_dram_parameter("wqT", [CH, CH], wdt, isOutput=False)
    wkT_d = nc.declare_dram_parameter("wkT", [CH, CH], wdt, isOutput=False)
    wvT_d = nc.declare_dram_parameter("wvT", [CH, CH], wdt, isOutput=False)
    wpP_d = nc.declare_dram_parameter("wpP", [P, HP, CH], BF16, isOutput=False)
    bqk_d = nc.declare_dram_parameter("bqk", [2, CH], F32, isOutput=False)
    brow_d = nc.declare_dram_parameter("brow", [2, CH], BF16, isOutput=False)
    estrips_d = nc.declare_dram_parameter("estrips", [HEADS, P, STRIP_W], BF16,
                                          isOutput=False)
    y_d = nc.declare_dram_parameter("y", [CH, NT], F32, isOutput=True)

    with tile.TileContext(nc) as tc, ExitStack() as ctx:
        singles = ctx.enter_context(tc.tile_pool(name="singles", bufs=1))
        work = ctx.enter_context(tc.tile_pool(name="work", bufs=2))
        strip_pool = ctx.enter_context(tc.tile_pool(name="strip_pool", bufs=2))
        at_pool = ctx.enter_context(tc.tile_pool(name="at_pool", bufs=10))
        # PSUM (8 banks): psA big (128,1024)f32 x2bufs = 4 banks, lives the
        # whole kernel (LN stats -> scores -> proj partials via same tag).
        psA = ctx.enter_context(tc.tile_pool(name="psA", bufs=2, space="PSUM"))

        # ---------- persistent SBUF ----------
        x_sb = singles.tile([P, CT, NT], F32)        # residual source
        xbf_sb = singles.tile([P, CT, NT], BF16)
        xn_sb = singles.tile([P, CT, NT],
                             FP8 if FP8_QKV else BF16)  # LN out (matmul input)
        qT_sb = singles.tile([P, CT, NT], BF16)      # (d part, t free)
        kT_sb = singles.tile([P, CT, NT], BF16)
        v_sb = singles.tile([P, TT, HEADS * VW], FP8 if FP8_AV else BF16)
        oT_sb = singles.tile([P, HP, NT], BF16)      # head pairs packed
        wpP_sb = singles.tile([P, HP, CH], BF16)
        bqk_sb = singles.tile([P, 2, CT], F32)       # per-partition bias cols q,k
        brow_sb = singles.tile([1, 2, CH], BF16)     # bv_eff, bp rows
        ones_mat = singles.tile([P, P], BF16)
        ones_row = singles.tile([1, NT], BF16)
        mu_sb = singles.tile([P, NT], F32)
        rs_sb = singles.tile([P, NT], F32)

        if USE_PBCAST:
            # partition_broadcast + gpsimd tensor_tensor both live in 'proxy'
            nc.gpsimd.load_library(library_config.proxy)
        nc.vector.memset(ones_mat[:], 1.0)
        nc.vector.memset(ones_row[:], 1.0)

        # v_aug per head (128 wide): even = [v(64) | 1 | 0*63], odd =
        # [0*32 | 1 | 0*31 | v(64)] — AV output rows are 0-63/64-127 with the
        # Z row at 64/32 (engine ops need start partition in {0,32,64}), and
        # the matmul writes a base-0 full-128 PSUM block.
        v_view = v_sb[:].rearrange("p tt (h w) -> p tt h w", w=VW)
        nc.gpsimd.memset(v_sb[:], 0.0)
        for h in range(HEADS):
            oc = HD if h % 2 == 0 else HD // 2
            nc.vector.memset(v_view[:, :, h, oc : oc + 1], 1.0)

        xbf_view = xbf_d.rearrange("(ct p) t -> p ct t", p=P)

        # ---------- phases 1+2: LayerNorm + QKV, pipelined by token half ----
        # Stats, the LN scalar chain, apply, and the Q/K/V projections are
        # all emitted per 512-token half so the second half's LN chain hides
        # under the first half's projection matmuls.
        QKV_DT = FP8 if FP8_QKV else BF16
        with tc.tile_pool(name="ln_pool", bufs=1) as lnp, \
             tc.tile_pool(name="psB", bufs=2, space="PSUM") as psB, \
             tc.tile_pool(name="wqkv_pool", bufs=1) as wp_pool:
            x2_sb = lnp.tile([P, CT, NT], BF16)
            x2_view = x2_d.rearrange("(ct p) t -> p ct t", p=P)
            # input DMAs split by token half (stats for half 0 need all four
            # channel tiles of that half); weight loads slot between the two
            # halves so QKV(ic0) never waits on them
            wqT_sb = wp_pool.tile([P, CT, CH], QKV_DT)
            wkT_sb = wp_pool.tile([P, CT, CH], QKV_DT)
            wvT_sb = wp_pool.tile([P, CT, CH], QKV_DT)
            sl0 = slice(0, 512)
            sl1 = slice(512, 1024)
            for ct in range(CT):
                nc.sync.dma_start(xbf_sb[:, ct, sl0], xbf_view[:, ct, sl0])
                nc.sync.dma_start(x2_sb[:, ct, sl0], x2_view[:, ct, sl0])
            nc.sync.dma_start(xbf_sb[:, :, sl1], xbf_view[:, :, sl1])
            nc.sync.dma_start(x2_sb[:, :, sl1], x2_view[:, :, sl1])
            # weights from the (idle-at-startup) ACT queue so they land in
            # parallel with the x-tile stream instead of serially behind it
            nc.scalar.dma_start(wqT_sb[:], wqT_d.rearrange("(ck p) d -> p ck d", p=P))
            nc.scalar.dma_start(wkT_sb[:], wkT_d.rearrange("(ck p) d -> p ck d", p=P))
            nc.scalar.dma_start(wvT_sb[:], wvT_d.rearrange("(ck p) d -> p ck d", p=P))
            nc.sync.dma_start(bqk_sb[:], bqk_d.rearrange("i (o p) -> p i o", p=P))
            nc.sync.dma_start(brow_sb[:], brow_d[None, :, :])
            nc.sync.dma_start(wpP_sb[:], wpP_d[:])
            nc.sync.dma_start(x_sb[:], x_d.rearrange("(ct p) t -> p ct t", p=P))

            def qkv_mms(ps, wsb, dsl, sl, v_swap=False):
                if FP8_QKV:
                    for cp in range(CT // 2):
                        c2 = slice(2 * cp, 2 * cp + 2)
                        lhsT = (xn_sb[:, c2, dsl] if v_swap else wsb[:, c2, dsl])
                        rhs = (wsb[:, c2, sl] if v_swap else xn_sb[:, c2, sl])
                        nc.tensor.matmul(ps[:], lhsT=lhsT, rhs=rhs,
                                         perf_mode=mybir.MatmulPerfMode.DoubleRow,
                                         start=(cp == 0),
                                         stop=(cp == CT // 2 - 1) and not v_swap)
                else:
                    for ck in range(CT):
                        lhsT = (xn_sb[:, ck, dsl] if v_swap else wsb[:, ck, dsl])
                        rhs = (wsb[:, ck, sl] if v_swap else xn_sb[:, ck, sl])
                        nc.tensor.matmul(ps[:], lhsT=lhsT, rhs=rhs,
                                         start=(ck == 0),
                                         stop=(ck == CT - 1) and not v_swap)

            # stats matmuls for BOTH token halves first, so the second half's
            # LN scalar chain hides under the first half's QKV matmuls
            stat_tiles = []
            for ic in range(IC):
                sl = slice(ic * 512, ic * 512 + 512)
                sum_ps = psA.tile([P, 512], F32, tag="big", name=f"sum_ps_{ic}")
                sq_ps = psA.tile([P, 512], F32, tag="big", name=f"sq_ps_{ic}")
                for ct in range(CT):
                    nc.tensor.matmul(sum_ps[:], lhsT=ones_mat[:],
                                     rhs=xbf_sb[:, ct, sl],
                                     start=(ct == 0), stop=(ct == CT - 1))
                    nc.tensor.matmul(sq_ps[:], lhsT=ones_mat[:],
                                     rhs=x2_sb[:, ct, sl],
                                     start=(ct == 0), stop=(ct == CT - 1))
                stat_tiles.append((sum_ps, sq_ps))

            for ic in range(IC):
                sl = slice(ic * 512, ic * 512 + 512)
                sum_ps, sq_ps = stat_tiles[ic]

                ve = lnp.tile([P, 512], F32, name=f"ve_{ic}", tag="ve", bufs=2)
                m2 = lnp.tile([P, 512], F32, name=f"m2_{ic}", tag="m2", bufs=2)
                lnv = lnp.tile([P, 512], F32, name=f"lnv_{ic}", tag="lnv", bufs=2)
                nc.scalar.activation(out=mu_sb[:, sl], in_=sum_ps[:],
                                     func=AF.Copy, scale=1.0 / CH)
                nc.vector.tensor_scalar(out=ve[:], in0=sq_ps[:], scalar1=1.0 / CH,
                                        scalar2=float(EPS), op0=OP.mult,
                                        op1=OP.add)
                nc.vector.tensor_tensor(out=m2[:], in0=mu_sb[:, sl],
                                        in1=mu_sb[:, sl], op=OP.mult)
                nc.vector.tensor_tensor(out=ve[:], in0=ve[:], in1=m2[:],
                                        op=OP.subtract)
                # rs = 1/sqrt(ve+eps) = exp(-0.5*ln(ve+eps)); Ln and Exp share
                # one ACT table set so no table reload happens mid-kernel.
                nc.scalar.activation(out=lnv[:], in_=ve[:], func=AF.Ln)
                nc.scalar.activation(out=rs_sb[:, sl], in_=lnv[:], func=AF.Exp,
                                     scale=-0.5)

                # LN apply reads the bf16 x copy — the fp32 x is only needed
                # for the residual add in phase 4 (its DMA is queued last).
                for ct in range(CT):
                    xc = lnp.tile([P, 512], F32, name=f"xc_{ic}_{ct}", tag="xc",
                                  bufs=2)
                    nc.gpsimd.tensor_tensor(out=xc[:], in0=xbf_sb[:, ct, sl],
                                            in1=mu_sb[:, sl], op=OP.subtract)
                    nc.vector.tensor_tensor(out=xn_sb[:, ct, sl], in0=xc[:],
                                            in1=rs_sb[:, sl], op=OP.mult)

                for dt in range(CT):
                    dsl = slice(dt * P, dt * P + P)
                    q_ps = psB.tile([P, 512], F32, tag="small")
                    qkv_mms(q_ps, wqT_sb, dsl, sl)
                    nc.vector.tensor_scalar_add(out=qT_sb[:, dt, sl],
                                                in0=q_ps[:],
                                                scalar1=bqk_sb[:, 0, dt : dt + 1])
                    k_ps = psB.tile([P, 512], F32, tag="small")
                    qkv_mms(k_ps, wkT_sb, dsl, sl)
                    nc.vector.tensor_scalar_add(out=kT_sb[:, dt, sl],
                                                in0=k_ps[:],
                                                scalar1=bqk_sb[:, 1, dt : dt + 1])

                for tt in range(4 * ic, 4 * ic + 4):
                    tsl = slice(tt * P, tt * P + P)
                    v_ps = psB.tile([P, 512], F32, tag="small")
                    qkv_mms(v_ps, wvT_sb, tsl, slice(0, CH), v_swap=True)
                    # + bv_eff (K=1 ones-row matmul)
                    nc.tensor.matmul(v_ps[:], lhsT=ones_row[:, :P],
                                     rhs=brow_sb[:, 0, :],
                                     start=False, stop=True)
                    # scatter per-head 64-wide blocks into the v-aug layout
                    # (even heads at cols 0-63 of their group, odd at 64-127)
                    vps_v = v_ps[:].rearrange("p (g hh d) -> p g hh d", hh=2, d=HD)
                    dst = v_view[:, tt]  # [p, h, VW]
                    dst_e = dst.rearrange("p (g hh) w -> p g hh w", hh=2)
                    nc.vector.tensor_copy(out=dst_e[:, :, 0, 0:HD],
                                          in_=vps_v[:, :, 0, :])
                    nc.vector.tensor_copy(out=dst_e[:, :, 1, HD:VW],
                                          in_=vps_v[:, :, 1, :])

        # ---------- phase 3: attention, software-pipelined per head ----------
        # Per head: scores (PE) -> exp (ACT) -> bias-mult (DVE) -> AV (PE,
        # interleaved 2 jt behind scores so the PE queue never waits on ACT).
        # The Z-normalize chain (1/Z = exp(-ln Z) on ACT, partition broadcast
        # via a DRAM round-trip on the DMA engines, multiply on DVE) has no
        # PE instructions and is emitted one head late so no engine queue
        # ever stalls on it.
        zdram = nc.dram_tensor("zscratch", [HEADS, NT], BF16, kind="Internal")
        zdram_f32 = nc.dram_tensor("zscratch32", [HEADS, NT], F32, kind="Internal")
        o_tiles = {}

        def z_start(h, via_pe=False):
            even = h % 2 == 0
            zrow = HD if even else HD // 2
            vlo = 0 if even else 64
            o_ps = o_tiles[h]
            if USE_DIV:
                # ship the raw Z row through DRAM; divide on DVE at finish
                zraw = work.tile([P, NT], F32, name=f"zraw_{h}", tag="zraw")
                nc.scalar.activation(out=zraw[zrow : zrow + 1, :],
                                     in_=o_ps[zrow : zrow + 1, :],
                                     func=AF.Identity)
                nc.scalar.dma_start(zdram_f32[h : h + 1, :],
                                    zraw[zrow : zrow + 1, :])
                zb = work.tile([P, NT], F32, name=f"zb_{h}", tag="zb")
                nc.gpsimd.dma_start(zb[vlo : vlo + HD, :],
                                    zdram_f32[h, :].partition_broadcast(HD))
                return zb
            # o_ps rows carry x16 when fp8 (v is x16); Z itself is true-scale.
            # zb = exp(-ln(16*Z)) = 1/(16Z) folds the x16 back out.
            zln = work.tile([P, NT], F32, name=f"zln_{h}", tag="zln")
            nc.scalar.activation(out=zln[zrow : zrow + 1, :],
                                 in_=o_ps[zrow : zrow + 1, :], func=AF.Ln,
                                 scale=FP8_SCALE if FP8_QKV else 1.0)
            zrec_bf = work.tile([P, NT], BF16, name=f"zrecb_{h}", tag="zrecb")
            nc.scalar.activation(out=zrec_bf[zrow : zrow + 1, :],
                                 in_=zln[zrow : zrow + 1, :], func=AF.Exp,
                                 scale=-1.0)
            zb = work.tile([P, NT], BF16, name=f"zb_{h}", tag="zb")
            if via_pe:
                # lower-latency replicate for the last head (PE idle then):
                # K=1 matmul + ACT copy instead of the DMA round-trip
                zrep_ps = psA.tile([P, NT], F32, tag="big", name=f"zrep_{h}")
                for ic in range(IC):
                    sl = slice(ic * 512, ic * 512 + 512)
                    nc.tensor.matmul(
                        zrep_ps[vlo : vlo + HD, sl],
                        lhsT=ones_mat[zrow : zrow + 1, :HD],
                        rhs=zrec_bf[zrow : zrow + 1, sl],
                        start=True, stop=True)
                nc.scalar.activation(out=zb[vlo : vlo + HD, :],
                                     in_=zrep_ps[vlo : vlo + HD, :],
                                     func=AF.Identity)
            else:
                # issue the round-trip DMAs from the producing/consuming
                # engine queues so the sync queue (strip prefetch) and these
                # never block each other
                nc.scalar.dma_start(zdram[h : h + 1, :],
                                    zrec_bf[zrow : zrow + 1, :])
                nc.gpsimd.dma_start(zb[vlo : vlo + HD, :],
                                    zdram[h, :].partition_broadcast(HD))
            return zb

        def z_finish(h, zb):
            even = h % 2 == 0
            vlo = 0 if even else 64
            nc.vector.tensor_tensor(
                out=oT_sb[vlo : vlo + HD, h // 2],
                in0=o_tiles[h][vlo : vlo + HD, :], in1=zb[vlo : vlo + HD, :],
                op=OP.divide if USE_DIV else OP.mult)

        with tc.tile_pool(name="ps_o", bufs=2, space="PSUM") as ps_o:
            strips = [None] * HEADS
            strips[0] = strip_pool.tile([P, STRIP_W], BF16, tag="strip",
                                        name="strip_0")
            nc.sync.dma_start(strips[0][:], estrips_d[0])
            zpend = None
            for h in range(HEADS):
                dtl = h // 2
                drow = HD * (h % 2)
                strip = strips[h]

                o_ps = ps_o.tile([P, NT], F32, tag="o", name=f"o_ps_{h}")
                o_tiles[h] = o_ps
                at_tiles = []

                def emit_av(jt):
                    # bf16 path: one jt per matmul
                    for ic in range(IC):
                        sl = slice(ic * 512, ic * 512 + 512)
                        nc.tensor.matmul(
                            o_ps[:, sl],
                            lhsT=v_sb[:, jt, h * VW : (h + 1) * VW],
                            rhs=at_tiles[jt][:, sl],
                            start=(jt == 0), stop=(jt == TT - 1),
                        )

                def emit_av_pair(jp):
                    # fp8 DoubleRow: jt pair (2jp, 2jp+1) contracted as K=256
                    for ic in range(IC):
                        sl = slice(ic * 512, ic * 512 + 512)
                        nc.tensor.matmul(
                            o_ps[:, sl],
                            lhsT=v_sb[:, 2 * jp : 2 * jp + 2,
                                      h * VW : (h + 1) * VW],
                            rhs=at_tiles[jp][:, :, sl],
                            perf_mode=mybir.MatmulPerfMode.DoubleRow,
                            start=(jp == 0), stop=(jp == TT // 2 - 1),
                        )

                for jt in range(TT):
                    s_ps = psA.tile([P, NT], F32, tag="big")
                    for ic in range(IC):
                        sl = slice(ic * 512, ic * 512 + 512)
                        nc.tensor.matmul(
                            s_ps[:, sl],
                            lhsT=kT_sb[drow : drow + HD, dtl, jt * P : jt * P + P],
                            rhs=qT_sb[drow : drow + HD, dtl, sl],
                            start=True, stop=True,
                        )
                    if FP8_AV:
                        if jt % 2 == 0:
                            at_tiles.append(at_pool.tile(
                                [P, 2, NT], FP8, name=f"aT_{h}_{jt//2}",
                                tag="aT"))
                        aslc = at_tiles[jt // 2][:, jt % 2, :]
                    else:
                        at_tiles.append(at_pool.tile(
                            [P, NT], BF16, name=f"aT_{h}_{jt}", tag="aT"))
                        aslc = at_tiles[jt][:, :]
                    if PROBE_NOEXP:
                        # timing probe: keep ACT+DVE idle (wrong numerics)
                        nc.vector.memset(aslc[:, 0:1], 0.001)
                    else:
                        # q and k both carry x16 when fp8 — undo via the
                        # activation's free affine scale; bias applied as a
                        # multiplicative exp(bias) strip, in place
                        nc.scalar.activation(out=aslc, in_=s_ps[:], func=AF.Exp,
                                             scale=1.0 / (FP8_SCALE ** 2)
                                             if FP8_QKV else 1.0)
                        off = (28 - 4 * jt) * 32
                        nc.vector.tensor_tensor(out=aslc, in0=aslc,
                                                in1=strip[:, off : off + NT],
                                                op=OP.mult)
                    if FP8_AV:
                        if jt >= 3 and jt % 2 == 1:
                            emit_av_pair((jt - 3) // 2)
                    elif jt >= 2:
                        emit_av(jt - 2)
                    if jt == 0 and h + 1 < HEADS:
                        # prefetch next head's exp(bias) strip
                        strips[h + 1] = strip_pool.tile(
                            [P, STRIP_W], BF16, tag="strip", name=f"strip_{h+1}")
                        nc.sync.dma_start(strips[h + 1][:], estrips_d[h + 1])
                    if jt == 1 and zpend is not None:
                        z_finish(*zpend)
                        zpend = None
                if FP8_AV:
                    emit_av_pair(TT // 2 - 2)
                    emit_av_pair(TT // 2 - 1)
                else:
                    emit_av(TT - 2)
                    emit_av(TT - 1)
                zpend = (h, z_start(
                    h, via_pe=(h == HEADS - 1 and not USE_DIV)))
            z_finish(*zpend)

        # ---------- phase 4: output projection + residual ----------
        for ct in range(CT):
            csl = slice(ct * P, ct * P + P)
            for icc in range(IC):
                sl = slice(icc * 512, icc * 512 + 512)
                y_ps = psA.tile([P, 512], F32, tag="big", name=f"y_ps_{ct}_{icc}")
                for hp in range(HP):
                    nc.tensor.matmul(y_ps[:], lhsT=wpP_sb[:, hp, csl],
                                     rhs=oT_sb[:, hp, sl],
                                     start=(hp == 0), stop=False)
                nc.tensor.matmul(y_ps[:], lhsT=brow_sb[:, 1, csl],
                                 rhs=ones_row[:, :512],
                                 start=False, stop=True)
                y_sb = work.tile([P, 512], F32, name=f"y_sb_{ct}_{icc}", tag="ysb")
                nc.vector.tensor_tensor(out=y_sb[:], in0=y_ps[:],
                                        in1=x_sb[:, ct, sl], op=OP.add)
                nc.sync.dma_start(y_d[csl, sl], y_sb[:])

    return nc


def _legalize_waits(nc, max_waits: int = 1):
    """Split multi-wait instructions into preceding same-engine NoOps.

    The TPB instruction encoding carries a single sync-wait slot and this
    walrus build refuses to legalize ("Too many sync wait commands"), so do
    it here: engines execute their queue in order, so a NoOp carrying one of
    the waits delays everything after it on that engine identically.
    """
    import orjson

    data = orjson.loads(mybir.module_to_json_bytes(nc.m))
    ctr = [0]

    def fix_block(block):
        out = []
        for inst in block.get("instructions", []):
            si = inst.get("sync_info") or {}
            waits = si.get("on_wait") or []
            if len(waits) > max_waits:
                for w in waits[max_waits:]:
                    ctr[0] += 1
                    nop = {
                        "name": f"I-WS{ctr[0]}",
                        "opcode": "NoOp",
                        "engine": inst["engine"],
                        "ins": [],
                        "outs": [],
                        "sync_info": {"on_wait": [w], "on_update": []},
                    }
                    if "debug" in inst:
                        nop["debug"] = inst["debug"]
                    out.append(nop)
                si = dict(si)
                si["on_wait"] = waits[:max_waits]
                inst["sync_info"] = si
            out.append(inst)
        block["instructions"] = out
        for b in block.get("blocks", []):
            fix_block(b)

    for fn in data["functions"]:
        for b in fn.get("blocks", []):
            fix_block(b)
    nc.m = mybir.module_from_json_bytes(orjson.dumps(data))
    return nc


_NC = None


def _host_prep(x, norm_w, norm_b, wq, bq, wk, bk, wv, bv, wp, bp, rel):
    scale = HD ** -0.5
    # fold LN affine + score scale into the projection weights (exact algebra)
    wq_eff = (wq * norm_w[None, :]) * scale
    bq_eff = (bq + wq @ norm_b) * scale
    wk_eff = wk * norm_w[None, :]
    bk_eff = bk + wk @ norm_b
    wv_eff = wv * norm_w[None, :]
    bv_eff = bv + wv @ norm_b

    if FP8_QKV:
        s16 = FP8_SCALE
        wdt = NPF8
    else:
        s16 = 1.0
        wdt = NPBF
    wqT = np.ascontiguousarray(wq_eff.T * s16).astype(wdt)
    wkT = np.ascontiguousarray(wk_eff.T * s16).astype(wdt)
    wvT = np.ascontiguousarray(wv_eff.T * s16).astype(wdt)
    # wp packed as head pairs: partitions 0-63 <- head 2hp, 64-127 <- head 2hp+1
    # under USE_DIV+fp8 the oT rows still carry x16 (raw-Z divide), folded out here
    wp_s = (1.0 / s16) if USE_DIV else 1.0
    wpP = np.ascontiguousarray(
        wp.T.reshape(HP, 2 * HD, CH).transpose(1, 0, 2) * wp_s
    ).astype(NPBF)

    bqk = (np.stack([bq_eff, bk_eff]) * s16).astype(np.float32)
    brow = np.stack([bv_eff * s16, bp]).astype(NPBF)
    estrips = np.exp(_build_strips(np.asarray(rel, np.float32))).astype(NPBF)

    shared = {
        "wqT": wqT, "wkT": wkT, "wvT": wvT, "wpP": wpP,
        "bqk": bqk, "brow": brow, "estrips": estrips,
    }
    in_maps = []
    for b in range(B):
        m = dict(shared)
        xb = np.ascontiguousarray(x[b].reshape(CH, NT)).astype(np.float32)
        m["x"] = xb
        m["xbf"] = xb.astype(NPBF)
        m["x2"] = (xb * xb).astype(NPBF)
        in_maps.append(m)
    return in_maps


def kernel(**inputs):
    global _NC
    if _NC is None:
        _NC = _legalize_waits(_build_nc())
    in_maps = _host_prep(**{k: np.asarray(v) for k, v in inputs.items()})
    res = run_bass_kernel_spmd(_NC, in_maps, list(range(B)))
    out = np.stack([res.results[b]["y"].reshape(CH, H, W) for b in range(B)])
    return out.astype(np.float32)


if __name__ == "__main__":
    nc = _build_nc()
    print("built OK")
